# revision 4
# baseline (speedup 1.0000x reference)
"""MetaLEAP edge scorer v3: SBUF-table ap_gather design.

Math (layer li):  b0 = psi_b + delta_w[li] + u[li]
  tabT[j, n]  = sum_c W[c, j] * x[n, c]          (PE, cols j<10 used)
      W[:, 0:4] = psi_w[0:64, :],  W[:, 4] = b0[0:64]
      W[:, 5:9] = psi_w[64:128, :], W[:, 9] = b0[64:128]
  s[e] = sum_{j<5} SF'[e,j] tabT[j, row_e]  +  sum_{j<5} SF'[e,j] tabT[5+j, col_e]
  y[e, h] = gamma[h] * leaky_relu(s[e])

Device plan per core (edges sharded, EC = E/8):
  Phase A (56 blocks of 2048 nodes): xT block (host-transposed, zero-padded
    to NP3=114688) -> bf16 cast -> 4 PE matmuls lhsT=[W|W] (64x32) at psum
    quadrants 0/32/64/96 -> fill [128, 512] -> tabT3[b] in HBM
    (node n = 2048 b + 512 m + k at tabT3[b, 32 m + j, k], col j < 10).
  Table rounds: 4 node-chunks of CH=28672; 16 (rowchunk,colchunk) buckets,
    mass-balanced into 2 rounds x 8 groups; SBUF tab row 16g+j holds tabT
    col j of group g's row-chunk (j<5) / col j-5 of its col-chunk (5<=j<10).
  Phase B per instruction-pair p (K=1024 edges x 8 groups):
    ap_gather (GPSIMD, 8 Q7 cores in parallel, per-group index lists) row+col
    -> DVE: pr/pc = gathered * SF'T -> bf16  [128, 1024]
    -> PE: 8 u-slices x 2 matmuls: lhsT = prod[:, 128u:128u+128] stationary,
       rhs = group masks [128, 8] moving -> psum[e, (p%4)*128+u*16+(0|8)+g]
       = per-group row/col partition sums; bank [128,512] = 4 pairs.
    -> DVE per bank: pair-reduce (row+col) -> leaky -> gamma broadcast
       -> yt [128, 2048] -> ydev[f].
Host does layout only: shard, bucket-sort, index/SF' packing, unpermute.
"""
import sys
if '/opt/trn_rl_repo' not in sys.path:
    sys.path.insert(0, '/opt/trn_rl_repo')

import numpy as np
import contextlib

import concourse.bacc as bacc
import concourse.mybir as mybir
from concourse.library_config import ap_gather as apg_lib
from concourse.bass_utils import run_bass_kernel_spmd

N = 100000
C = 64
E = 1600000
H = 8
NEG = 0.01
NCORES = 8
CH = 26624            # SBUF table size = max chunk (13 * 2048)
NCH = 4
CB = (0, 26624, 51200, 75776, 100352)   # chunk boundaries (2048-aligned)
FB = (0, 13, 25, 37)                    # chunk start in 2048-node blocks
NFC = (13, 12, 12, 12)                  # blocks per chunk
NP3 = CB[4]           # 100352 (padded)
K = 1024              # edges per instruction per group
XW = 2048             # phase-A node block
EC = E // NCORES


def build_program3(tiles, rounds, nrep=1):
    """tiles[q]: instruction-pairs in round q; rounds[q][g]: bucket id
    (rc*NCH+cc) whose table group g holds during round q."""
    NI = sum(tiles)
    NF = (NI + 3) // 4              # psum-bank fills (4 pairs each)
    NRND = len(tiles)
    rbase = np.concatenate([[0], np.cumsum(tiles)]).astype(int)

    NBA = NP3 // XW                 # 56 phase-A blocks == fills
    IW = K // 16

    nc = bacc.Bacc("TRN2", target_bir_lowering=False, debug=False,
                   num_devices=NCORES, num_swdge_queues=1,
                   detect_race_conditions=False)

    xt_in = nc.dram_tensor("xt_in", [C, NP3], mybir.dt.float32,
                           kind="ExternalInput")
    wext = nc.dram_tensor("wext", [C, 32], mybir.dt.bfloat16,
                          kind="ExternalInput")
    mks = nc.dram_tensor("mks", [128, 16], mybir.dt.bfloat16,
                         kind="ExternalInput")
    gma = nc.dram_tensor("gma", [128, H], mybir.dt.float32,
                         kind="ExternalInput")
    idxr = nc.dram_tensor("idxr", [NI, 128, IW], mybir.dt.int16,
                          kind="ExternalInput")
    idxc = nc.dram_tensor("idxc", [NI, 128, IW], mybir.dt.int16,
                          kind="ExternalInput")
    sft = nc.dram_tensor("sft", [NI, 128, K], mybir.dt.float32,
                         kind="ExternalInput")
    ydev = nc.dram_tensor("ydev", [NF, 128, 256 * H], mybir.dt.float32,
                          kind="ExternalOutput")
    tabT = nc.dram_tensor("tabT", [NBA, 128, 512], mybir.dt.float32)

    with contextlib.ExitStack() as ctx:
        e = ctx.enter_context
        xs = [e(nc.sbuf_tensor(f"xs{i}", [128, XW], mybir.dt.float32))
              for i in range(4)]
        xb = [e(nc.sbuf_tensor(f"xb{i}", [128, XW], mybir.dt.bfloat16))
              for i in range(2)]
        wt = e(nc.sbuf_tensor("wt", [128, 32], mybir.dt.bfloat16))
        mk = e(nc.sbuf_tensor("mk", [128, 16], mybir.dt.bfloat16))
        gm = e(nc.sbuf_tensor("gm", [128, H], mybir.dt.float32))
        tstg = [e(nc.sbuf_tensor(f"tstg{i}", [128, 512], mybir.dt.float32))
                for i in range(2)]
        tab = e(nc.sbuf_tensor("tab", [128, CH], mybir.dt.float32))
        idr = [e(nc.sbuf_tensor(f"idr{i}", [128, IW], mybir.dt.int16))
               for i in range(2)]
        idc = [e(nc.sbuf_tensor(f"idc{i}", [128, IW], mybir.dt.int16))
               for i in range(2)]
        sfb = [e(nc.sbuf_tensor(f"sfb{i}", [128, K], mybir.dt.float32))
               for i in range(2)]
        gr = [e(nc.sbuf_tensor(f"gr{i}", [128, K], mybir.dt.float32))
              for i in range(2)]
        gc = [e(nc.sbuf_tensor(f"gc{i}", [128, K], mybir.dt.float32))
              for i in range(2)]
        pr = e(nc.sbuf_tensor("pr", [128, K], mybir.dt.bfloat16))
        pc = e(nc.sbuf_tensor("pc", [128, K], mybir.dt.bfloat16))
        sg = e(nc.sbuf_tensor("sg", [128, 256], mybir.dt.float32))
        ys = e(nc.sbuf_tensor("ys", [128, 256], mybir.dt.float32))
        yt = e(nc.sbuf_tensor("yt", [128, 256 * H], mybir.dt.float32))
        psA = [e(nc.psum_tensor(f"psA{i}", [128, 512], mybir.dt.float32))
               for i in range(2)]
        psB = [e(nc.psum_tensor(f"psB{i}", [128, 512], mybir.dt.float32))
               for i in range(2)]
        s_ini = e(nc.semaphore("s_ini"))
        s_xt = e(nc.semaphore("s_xt"))
        s_cx = e(nc.semaphore("s_cx"))
        s_pa = e(nc.semaphore("s_pa"))
        s_tc = e(nc.semaphore("s_tc"))
        s_tw = e(nc.semaphore("s_tw"))
        s_tb = e(nc.semaphore("s_tb"))
        s_ix = e(nc.semaphore("s_ix"))
        s_gg = e(nc.semaphore("s_gg"))
        s_pp = e(nc.semaphore("s_pp"))
        s_mm = e(nc.semaphore("s_mm"))
        s_yv = e(nc.semaphore("s_yv"))
        s_yd = e(nc.semaphore("s_yd"))
        s_zt = e(nc.semaphore("s_zt"))
        s_f = e(nc.semaphore("s_f"))
        block = e(nc.Block())

        def roundld(sy, rep, q):
            sy.wait_ge(s_zt, rep + 1)                   # tab memset done
            gge = rep * 2 * NI + 2 * int(rbase[q])
            if rep > 0 or q > 0:
                sy.wait_ge(s_gg, gge)                   # prior gathers done
            src = tabT[:].rearrange("F (m j) k -> j F m k", j=32)
            for g in range(8):
                b = rounds[q][g]
                rc, cc = divmod(b, NCH)
                sy.wait_ge(s_tw, 16 * (rep * NBA + FB[rc] + NFC[rc]))
                sy.dma_start(
                    tab[16 * g:16 * g + 5, :NFC[rc] * 2048].rearrange(
                        "p (F m k) -> p F m k", F=NFC[rc], m=4),
                    src[0:5, FB[rc]:FB[rc] + NFC[rc]],
                ).then_inc(s_tb, 16)
                sy.wait_ge(s_tw, 16 * (rep * NBA + FB[cc] + NFC[cc]))
                sy.dma_start(
                    tab[16 * g + 5:16 * g + 10, :NFC[cc] * 2048].rearrange(
                        "p (F m k) -> p F m k", F=NFC[cc], m=4),
                    src[5:10, FB[cc]:FB[cc] + NFC[cc]],
                ).then_inc(s_tb, 16)

        def loads(qq_, rep, p):
            qq_.dma_start(idr[p % 2][:], idxr[p]).then_inc(s_ix, 16)
            qq_.dma_start(idc[p % 2][:], idxc[p]).then_inc(s_ix, 16)
            qq_.dma_start(sfb[p % 2][:], sft[p]).then_inc(s_ix, 16)

        @block.sync
        def _(sy):
            sy.dma_start(wt[0:C, :], wext[:]).then_inc(s_ini, 16)
            sy.dma_start(mk[:], mks[:]).then_inc(s_ini, 16)
            sy.dma_start(gm[:], gma[:]).then_inc(s_ini, 16)
            for rep in range(nrep):
                if rep > 0:
                    sy.wait_ge(s_cx, NBA * rep)
                    sy.wait_ge(s_pa, 4 * NBA * rep)
                    sy.wait_ge(s_tc, NBA * rep)
                    sy.wait_ge(s_tw, 16 * NBA * rep)
                    sy.wait_ge(s_tb, 256 * NRND * rep)
                    sy.wait_ge(s_ix, 48 * NI * rep)
                    sy.wait_ge(s_gg, 2 * NI * rep)
                    sy.wait_ge(s_pp, 2 * NI * rep)
                    sy.wait_ge(s_mm, NI * rep)
                    sy.wait_ge(s_yv, NF * rep)
                    sy.wait_ge(s_yd, 16 * NF * rep)
                    sy.wait_ge(s_zt, rep)
                    sy.sem_inc(s_f, 1)
                for b in range(NBA):
                    if rep * NBA + b >= 4:
                        sy.wait_ge(s_cx, rep * NBA + b - 3)
                    sy.dma_start(xs[b % 4][0:C, :],
                                 xt_in[:, b * XW:(b + 1) * XW]
                                 ).then_inc(s_xt, 16)
                    if b >= 1:
                        F = b - 1
                        sy.wait_ge(s_tc, rep * NBA + F + 1)
                        sy.dma_start(tabT[F],
                                     tstg[F % 2][:]).then_inc(s_tw, 16)
                sy.wait_ge(s_tc, rep * NBA + NBA)
                sy.dma_start(tabT[NBA - 1],
                             tstg[(NBA - 1) % 2][:]).then_inc(s_tw, 16)
                for q in range(NRND):
                    roundld(sy, rep, q)

        @block.scalar
        def _(sc):
            sc.wait_ge(s_ini, 48)
            for rep in range(nrep):
                if rep > 0:
                    sc.wait_ge(s_f, rep)
                # phase A casts
                for b in range(NBA):
                    sc.wait_ge(s_xt, 16 * (rep * NBA + b + 1))
                    if b >= 2:
                        sc.wait_ge(s_pa, 4 * (rep * NBA + b - 1))
                    sc.activation(xb[b % 2][0:C, :], xs[b % 4][0:C, :],
                                  mybir.ActivationFunctionType.Copy
                                  ).then_inc(s_cx, 1)
                # phase B edge loads + output stores
                loads(sc, rep, 0)
                if NI > 1:
                    loads(sc, rep, 1)
                for p in range(NI):
                    gp_ = rep * NI + p
                    if p + 2 < NI:
                        sc.wait_ge(s_gg, 2 * (gp_ + 1))
                        sc.wait_ge(s_pp, 2 * (gp_ + 1))
                        loads(sc, rep, p + 2)
                    if (p + 1) % 4 == 0 or p == NI - 1:
                        f = p // 4
                        sc.wait_ge(s_yv, rep * NF + f + 1)
                        sc.dma_start(ydev[f], yt[:]).then_inc(s_yd, 16)

        @block.tensor
        def _(te):
            te.wait_ge(s_ini, 48)
            for rep in range(nrep):
                if rep > 0:
                    te.wait_ge(s_f, rep)
                for b in range(NBA):
                    te.wait_ge(s_cx, rep * NBA + b + 1)
                    if b >= 2:
                        te.wait_ge(s_tc, rep * NBA + b - 1)
                    for m in range(4):
                        te.matmul(psA[b % 2][32 * m:32 * m + 32, :],
                                  wt[0:C, :],
                                  xb[b % 2][0:C, m * 512:(m + 1) * 512],
                                  start=True, stop=True,
                                  tile_position=(0, 32 * m)).then_inc(s_pa, 1)
                for p in range(NI):
                    gp_ = rep * NI + p
                    te.wait_ge(s_pp, 2 * (gp_ + 1))
                    f = p // 4
                    if p % 4 == 0 and f >= 2:
                        te.wait_ge(s_yv, rep * NF + f - 1)
                    bank = f % 2
                    last = None
                    for uu in range(8):
                        o = (p % 4) * 128 + uu * 16
                        te.matmul(psB[bank][:, o:o + 8],
                                  pr[:, uu * 128:(uu + 1) * 128],
                                  mk[:, 0:8], start=True, stop=True)
                        last = te.matmul(psB[bank][:, o + 8:o + 16],
                                         pc[:, uu * 128:(uu + 1) * 128],
                                         mk[:, 8:16], start=True, stop=True)
                    last.then_inc(s_mm, 1)

        @block.vector
        def _(ve):
            ve.wait_ge(s_ini, 48)
            for rep in range(nrep):
                if rep > 0:
                    ve.wait_ge(s_f, rep)
                ve.memset(tab[:], 0.0).then_inc(s_zt, 1)
                for b in range(NBA):
                    ve.wait_ge(s_pa, 4 * (rep * NBA + b) + 4)
                    if b >= 2:
                        ve.wait_ge(s_tw, 16 * (rep * NBA + b - 1))
                    ve.tensor_copy(tstg[b % 2][:],
                                   psA[b % 2][:]).then_inc(s_tc, 1)
                for p in range(NI):
                    gp_ = rep * NI + p
                    ve.wait_ge(s_gg, 2 * (gp_ + 1))
                    ve.wait_ge(s_ix, 48 * (gp_ + 1))
                    if p >= 1:
                        ve.wait_ge(s_mm, gp_)
                    ve.tensor_tensor(out=pr[:], in0=gr[p % 2][:],
                                     in1=sfb[p % 2][:],
                                     op=mybir.AluOpType.mult).then_inc(s_pp, 1)
                    ve.tensor_tensor(out=pc[:], in0=gc[p % 2][:],
                                     in1=sfb[p % 2][:],
                                     op=mybir.AluOpType.mult).then_inc(s_pp, 1)
                    if (p + 1) % 4 == 0 or p == NI - 1:
                        f = p // 4
                        ve.wait_ge(s_mm, rep * NI + min(4 * (f + 1), NI))
                        if f >= 1:
                            ve.wait_ge(s_yd, 16 * (rep * NF + f))
                        ve.tensor_reduce(
                            out=sg[:].rearrange("p (pb u g) -> p pb u g",
                                                pb=4, u=8),
                            in_=psB[f % 2][:].rearrange(
                                "p (pb u t g) -> p pb u g t",
                                pb=4, u=8, t=2, g=8),
                            axis=mybir.AxisListType.X,
                            op=mybir.AluOpType.add)
                        ve.scalar_tensor_tensor(
                            out=ys[:], in0=sg[:], scalar=NEG,
                            in1=sg[:], op0=mybir.AluOpType.mult,
                            op1=mybir.AluOpType.max)
                        ve.tensor_tensor(
                            out=yt[:].rearrange("p (k h) -> p k h", h=H),
                            in0=ys[:].unsqueeze(2).broadcast_to([128, 256, H]),
                            in1=gm[:].unsqueeze(1).broadcast_to([128, 256, H]),
                            op=mybir.AluOpType.mult).then_inc(s_yv, 1)

        @block.gpsimd
        def _(gp):
            gp.load_library(apg_lib)
            for rep in range(nrep):
                if rep > 0:
                    gp.wait_ge(s_f, rep)
                q = 0
                for p in range(NI):
                    gp_ = rep * NI + p
                    while q < NRND and p == int(rbase[q]):
                        gp.wait_ge(s_tb, 256 * (rep * NRND + q) + 256)
                        q += 1
                    gp.wait_ge(s_ix, 48 * (gp_ + 1))
                    if p >= 2:
                        gp.wait_ge(s_pp, 2 * (gp_ - 1))
                    gp.ap_gather(
                        gr[p % 2][:].rearrange("p (k d) -> p k d", d=1),
                        tab[:].rearrange("p (n d) -> p n d", d=1),
                        idr[p % 2][:], 128, CH, 1, K).then_inc(s_gg, 1)
                    gp.ap_gather(
                        gc[p % 2][:].rearrange("p (k d) -> p k d", d=1),
                        tab[:].rearrange("p (n d) -> p n d", d=1),
                        idc[p % 2][:], 128, CH, 1, K).then_inc(s_gg, 1)

    nc.compile()
    return nc


def plan_rounds(cnts):
    """cnts [NCORES, 16] -> (tiles, rounds): 8 largest buckets (by max core
    count) in round 0, rest in round 1."""
    mx = cnts.max(axis=0)
    order = np.argsort(-mx, kind='stable')
    rounds = [list(int(b) for b in order[0:8]),
              list(int(b) for b in order[8:16])]
    tiles = [int(max(1, -(-int(mx[list(r)].max()) // K)))
             for r in rounds]
    return tiles, rounds


def prep_inputs3(x, edge_index, structural_features, layer_idx,
                 psi_w, psi_b, delta_w, u, gamma_h):
    import ml_dtypes
    li = int(layer_idx)
    b0 = np.asarray(psi_b + delta_w[li] + u[li], np.float32)      # [128]
    psi_w = np.asarray(psi_w, np.float32)
    W = np.zeros((C, 16), dtype=np.float32)
    W[:, 0:4] = psi_w[0:C]
    W[:, 4] = b0[0:C]
    W[:, 5:9] = psi_w[C:2 * C]
    W[:, 9] = b0[C:2 * C]
    wext = np.concatenate([W, W], axis=1).astype(ml_dtypes.bfloat16)
    mks = np.zeros((128, 16), dtype=np.float32)
    for g in range(8):
        mks[16 * g:16 * g + 5, g] = 1.0
        mks[16 * g + 5:16 * g + 10, 8 + g] = 1.0
    mks = mks.astype(ml_dtypes.bfloat16)
    gma = np.tile(np.asarray(gamma_h[li], np.float32)[None, :], (128, 1))
    xT = np.zeros((C, NP3), dtype=np.float32)
    xT[:, :N] = np.asarray(x, np.float32).T

    row = np.asarray(edge_index[0], np.int64)
    col = np.asarray(edge_index[1], np.int64)
    sfp = np.concatenate([np.asarray(structural_features, np.float32),
                          np.ones((E, 1), np.float32)], axis=1)      # [E, 5]

    cb = np.asarray(CB[1:4], np.int64)
    rch = np.searchsorted(cb, row, side='right')
    cch = np.searchsorted(cb, col, side='right')
    cbase = np.asarray(CB[:4], np.int64)
    bucket = rch * NCH + cch
    orders, cnts = [], np.zeros((NCORES, 16), dtype=np.int64)
    for c in range(NCORES):
        sl = slice(c * EC, (c + 1) * EC)
        orders.append(np.argsort(bucket[sl], kind='stable') + c * EC)
        cnts[c] = np.bincount(bucket[sl], minlength=16)
    tiles, rounds = plan_rounds(cnts)
    NI = sum(tiles)
    NF = (NI + 3) // 4

    in_maps, eid_all = [], []
    for c in range(NCORES):
        order = orders[c]
        boff = np.concatenate([[0], np.cumsum(cnts[c])])
        ids = np.full((NI, 8, K), -1, dtype=np.int64)
        for qr in range(2):
            p0 = int(0 if qr == 0 else tiles[0])
            for g in range(8):
                b = int(rounds[qr][g])
                lo, hi = int(boff[b]), int(boff[b + 1])
                buf = np.full(tiles[qr] * K, -1, dtype=np.int64)
                buf[:hi - lo] = order[lo:hi]
                ids[p0:p0 + tiles[qr], g, :] = buf.reshape(tiles[qr], K)
        valid = ids >= 0
        idsz = np.where(valid, ids, 0)
        rl = np.where(valid, row[idsz] - cbase[rch[idsz]], 0).astype(np.int16)
        cl = np.where(valid, col[idsz] - cbase[cch[idsz]], 0).astype(np.int16)

        def wrap(a):   # [NI, 8, K] -> [NI, 128, K//16]
            return np.ascontiguousarray(
                a.reshape(NI, 8, K // 16, 16).transpose(0, 1, 3, 2)
                .reshape(NI, 128, K // 16))
        sfv = np.where(valid[..., None], sfp[idsz], 0.0)   # [NI, 8, K, 5]
        sftd = np.zeros((NI, 8, 16, K), dtype=np.float32)
        sftd[:, :, 0:5, :] = sfv.transpose(0, 1, 3, 2)
        sftd[:, :, 5:10, :] = sfv.transpose(0, 1, 3, 2)
        in_maps.append({
            "xt_in": xT, "wext": wext, "mks": mks, "gma": gma,
            "idxr": wrap(rl), "idxc": wrap(cl),
            "sft": np.ascontiguousarray(sftd.reshape(NI, 128, K)),
        })
        # edge (p, g, u*128 + e_p) -> yd[f=p//4, e_p, (p%4, u, g)]
        NIp = NF * 4
        idsp = np.full((NIp, 8, K), -1, dtype=np.int64)
        idsp[:NI] = ids
        eo = (idsp.reshape(NF, 4, 8, 8, 128)      # [f, pb, g, u, e_p]
              .transpose(0, 4, 1, 3, 2))          # [f, e_p, pb, u, g]
        eid_all.append(np.ascontiguousarray(eo))
    return in_maps, eid_all, (tiles, rounds)


def unshard3(results, eid_all):
    y = np.empty((E, H), dtype=np.float32)
    for c in range(NCORES):
        yd = np.asarray(results[c]["ydev"])       # [NF, 128, 2048]
        ids = eid_all[c].reshape(-1)              # [f, e_p, pb, u, g]
        blk = yd.reshape(-1, H)
        v = ids >= 0
        y[ids[v]] = blk[v]
    return y


_CACHE = {}


def kernel(**inputs):
    in_maps, eid_all, (tiles, rounds) = prep_inputs3(**inputs)
    key = (tuple(tiles), tuple(tuple(r) for r in rounds))
    if key not in _CACHE:
        _CACHE[key] = build_program3(tiles, rounds)
    res = run_bass_kernel_spmd(_CACHE[key], in_maps,
                               core_ids=list(range(NCORES)))
    return unshard3(res.results, eid_all)


# revision 5
# speedup vs baseline: 1.0538x; 1.0538x over previous
"""MetaLEAP edge scorer v3: SBUF-table ap_gather design.

Math (layer li):  b0 = psi_b + delta_w[li] + u[li]
  tabT[j, n]  = sum_c W[c, j] * x[n, c]          (PE, cols j<10 used)
      W[:, 0:4] = psi_w[0:64, :],  W[:, 4] = b0[0:64]
      W[:, 5:9] = psi_w[64:128, :], W[:, 9] = b0[64:128]
  s[e] = sum_{j<5} SF'[e,j] tabT[j, row_e]  +  sum_{j<5} SF'[e,j] tabT[5+j, col_e]
  y[e, h] = gamma[h] * leaky_relu(s[e])

Device plan per core (edges sharded, EC = E/8):
  Phase A (56 blocks of 2048 nodes): xT block (host-transposed, zero-padded
    to NP3=114688) -> bf16 cast -> 4 PE matmuls lhsT=[W|W] (64x32) at psum
    quadrants 0/32/64/96 -> fill [128, 512] -> tabT3[b] in HBM
    (node n = 2048 b + 512 m + k at tabT3[b, 32 m + j, k], col j < 10).
  Table rounds: 4 node-chunks of CH=28672; 16 (rowchunk,colchunk) buckets,
    mass-balanced into 2 rounds x 8 groups; SBUF tab row 16g+j holds tabT
    col j of group g's row-chunk (j<5) / col j-5 of its col-chunk (5<=j<10).
  Phase B per instruction-pair p (K=1024 edges x 8 groups):
    ap_gather (GPSIMD, 8 Q7 cores in parallel, per-group index lists) row+col
    -> DVE: pr/pc = gathered * SF'T -> bf16  [128, 1024]
    -> PE: 8 u-slices x 2 matmuls: lhsT = prod[:, 128u:128u+128] stationary,
       rhs = group masks [128, 8] moving -> psum[e, (p%4)*128+u*16+(0|8)+g]
       = per-group row/col partition sums; bank [128,512] = 4 pairs.
    -> DVE per bank: pair-reduce (row+col) -> leaky -> gamma broadcast
       -> yt [128, 2048] -> ydev[f].
Host does layout only: shard, bucket-sort, index/SF' packing, unpermute.
"""
import sys
if '/opt/trn_rl_repo' not in sys.path:
    sys.path.insert(0, '/opt/trn_rl_repo')

import numpy as np
import contextlib

import concourse.bacc as bacc
import concourse.mybir as mybir
from concourse.library_config import ap_gather as apg_lib
from concourse.bass_utils import run_bass_kernel_spmd

N = 100000
C = 64
E = 1600000
H = 8
NEG = 0.01
NCORES = 8
CH = 26624            # SBUF table size = max chunk (13 * 2048)
NCH = 4
CB = (0, 26624, 51200, 75776, 100352)   # chunk boundaries (2048-aligned)
FB = (0, 13, 25, 37)                    # chunk start in 2048-node blocks
NFC = (13, 12, 12, 12)                  # blocks per chunk
NP3 = CB[4]           # 100352 (padded)
K = 1024              # edges per instruction per group
XW = 2048             # phase-A node block
EC = E // NCORES


def build_program3(tiles, rounds, nrep=1):
    """tiles[q]: instruction-pairs in round q; rounds[q][g]: bucket id
    (rc*NCH+cc) whose table group g holds during round q."""
    NI = sum(tiles)
    NF = (NI + 3) // 4              # psum-bank fills (4 pairs each)
    NRND = len(tiles)
    rbase = np.concatenate([[0], np.cumsum(tiles)]).astype(int)

    NBA = NP3 // XW                 # 56 phase-A blocks == fills
    IW = K // 16

    nc = bacc.Bacc("TRN2", target_bir_lowering=False, debug=False,
                   num_devices=NCORES, num_swdge_queues=1,
                   detect_race_conditions=False)

    xt_in = nc.dram_tensor("xt_in", [C, NP3], mybir.dt.float32,
                           kind="ExternalInput")
    wext = nc.dram_tensor("wext", [C, 32], mybir.dt.bfloat16,
                          kind="ExternalInput")
    mks = nc.dram_tensor("mks", [128, 16], mybir.dt.bfloat16,
                         kind="ExternalInput")
    gma = nc.dram_tensor("gma", [128, H], mybir.dt.float32,
                         kind="ExternalInput")
    idxr = nc.dram_tensor("idxr", [NI, 128, IW], mybir.dt.int16,
                          kind="ExternalInput")
    idxc = nc.dram_tensor("idxc", [NI, 128, IW], mybir.dt.int16,
                          kind="ExternalInput")
    sft = nc.dram_tensor("sft", [NI, 128, K], mybir.dt.float32,
                         kind="ExternalInput")
    ydev = nc.dram_tensor("ydev", [NF, 128, 256 * H], mybir.dt.float32,
                          kind="ExternalOutput")
    tabT = nc.dram_tensor("tabT", [16, NP3], mybir.dt.float32)

    with contextlib.ExitStack() as ctx:
        e = ctx.enter_context
        xs = [e(nc.sbuf_tensor(f"xs{i}", [128, XW], mybir.dt.float32))
              for i in range(4)]
        xb = [e(nc.sbuf_tensor(f"xb{i}", [128, XW], mybir.dt.bfloat16))
              for i in range(2)]
        wt = e(nc.sbuf_tensor("wt", [128, 32], mybir.dt.bfloat16))
        mk = e(nc.sbuf_tensor("mk", [128, 16], mybir.dt.bfloat16))
        gm = e(nc.sbuf_tensor("gm", [128, H], mybir.dt.float32))
        tstg = [e(nc.sbuf_tensor(f"tstg{i}", [128, 512], mybir.dt.float32))
                for i in range(2)]
        tab = e(nc.sbuf_tensor("tab", [128, CH], mybir.dt.float32))
        idr = [e(nc.sbuf_tensor(f"idr{i}", [128, IW], mybir.dt.int16))
               for i in range(3)]
        idc = [e(nc.sbuf_tensor(f"idc{i}", [128, IW], mybir.dt.int16))
               for i in range(3)]
        sfb = [e(nc.sbuf_tensor(f"sfb{i}", [128, K], mybir.dt.float32))
               for i in range(3)]
        gr = [e(nc.sbuf_tensor(f"gr{i}", [128, K], mybir.dt.float32))
              for i in range(2)]
        gc = [e(nc.sbuf_tensor(f"gc{i}", [128, K], mybir.dt.float32))
              for i in range(2)]
        pr = e(nc.sbuf_tensor("pr", [128, K], mybir.dt.bfloat16))
        pc = e(nc.sbuf_tensor("pc", [128, K], mybir.dt.bfloat16))
        sg = e(nc.sbuf_tensor("sg", [128, 256], mybir.dt.float32))
        ys = e(nc.sbuf_tensor("ys", [128, 256], mybir.dt.float32))
        yt = e(nc.sbuf_tensor("yt", [128, 256 * H], mybir.dt.float32))
        psA = [e(nc.psum_tensor(f"psA{i}", [128, 512], mybir.dt.float32))
               for i in range(2)]
        psB = [e(nc.psum_tensor(f"psB{i}", [128, 512], mybir.dt.float32))
               for i in range(2)]
        s_ini = e(nc.semaphore("s_ini"))
        s_xt = e(nc.semaphore("s_xt"))
        s_cx = e(nc.semaphore("s_cx"))
        s_pa = e(nc.semaphore("s_pa"))
        s_tc = e(nc.semaphore("s_tc"))
        s_tw = e(nc.semaphore("s_tw"))
        s_tb = e(nc.semaphore("s_tb"))
        s_ix = e(nc.semaphore("s_ix"))
        s_gg = e(nc.semaphore("s_gg"))
        s_pp = e(nc.semaphore("s_pp"))
        s_mm = e(nc.semaphore("s_mm"))
        s_yv = e(nc.semaphore("s_yv"))
        s_yd = e(nc.semaphore("s_yd"))
        s_zt = e(nc.semaphore("s_zt"))
        s_f = e(nc.semaphore("s_f"))
        block = e(nc.Block())

        def roundld(sy, rep, q):
            sy.wait_ge(s_zt, rep + 1)                   # tab memset done
            gge = rep * 2 * NI + 2 * int(rbase[q])
            if rep > 0 or q > 0:
                sy.wait_ge(s_gg, gge)                   # prior gathers done
            for g in range(8):
                b = rounds[q][g]
                rc, cc = divmod(b, NCH)
                wr = NFC[rc] * 2048
                wc = NFC[cc] * 2048
                sy.wait_ge(s_tw, 64 * (rep * NBA + FB[rc] + NFC[rc]))
                sy.dma_start(tab[16 * g:16 * g + 5, :wr],
                             tabT[0:5, CB[rc]:CB[rc] + wr]).then_inc(s_tb, 16)
                sy.wait_ge(s_tw, 64 * (rep * NBA + FB[cc] + NFC[cc]))
                sy.dma_start(tab[16 * g + 5:16 * g + 10, :wc],
                             tabT[5:10, CB[cc]:CB[cc] + wc]).then_inc(s_tb, 16)

        def loads(qq_, rep, p):
            qq_.dma_start(idr[p % 3][:], idxr[p]).then_inc(s_ix, 16)
            qq_.dma_start(idc[p % 3][:], idxc[p]).then_inc(s_ix, 16)
            qq_.dma_start(sfb[p % 3][:], sft[p]).then_inc(s_ix, 16)

        @block.sync
        def _(sy):
            sy.dma_start(wt[0:C, :], wext[:]).then_inc(s_ini, 16)
            sy.dma_start(mk[:], mks[:]).then_inc(s_ini, 16)
            sy.dma_start(gm[:], gma[:]).then_inc(s_ini, 16)
            for rep in range(nrep):
                if rep > 0:
                    sy.wait_ge(s_cx, NBA * rep)
                    sy.wait_ge(s_pa, 4 * NBA * rep)
                    sy.wait_ge(s_tc, NBA * rep)
                    sy.wait_ge(s_tw, 64 * NBA * rep)
                    sy.wait_ge(s_tb, 256 * NRND * rep)
                    sy.wait_ge(s_ix, 48 * NI * rep)
                    sy.wait_ge(s_gg, 2 * NI * rep)
                    sy.wait_ge(s_pp, 2 * NI * rep)
                    sy.wait_ge(s_mm, NI * rep)
                    sy.wait_ge(s_yv, NF * rep)
                    sy.wait_ge(s_yd, 16 * NF * rep)
                    sy.wait_ge(s_zt, rep)
                    sy.sem_inc(s_f, 1)
                for b in range(NBA):
                    if rep * NBA + b >= 4:
                        sy.wait_ge(s_cx, rep * NBA + b - 3)
                    sy.dma_start(xs[b % 4][0:C, :],
                                 xt_in[:, b * XW:(b + 1) * XW]
                                 ).then_inc(s_xt, 16)
                    if b >= 1:
                        F = b - 1
                        sy.wait_ge(s_tc, rep * NBA + F + 1)
                        for mm in range(4):
                            sy.dma_start(
                                tabT[0:10, F * 2048 + mm * 512:
                                     F * 2048 + mm * 512 + 512],
                                tstg[F % 2][32 * mm:32 * mm + 10, :],
                            ).then_inc(s_tw, 16)
                sy.wait_ge(s_tc, rep * NBA + NBA)
                for mm in range(4):
                    F = NBA - 1
                    sy.dma_start(
                        tabT[0:10, F * 2048 + mm * 512:
                             F * 2048 + mm * 512 + 512],
                        tstg[F % 2][32 * mm:32 * mm + 10, :],
                    ).then_inc(s_tw, 16)
                for q in range(NRND):
                    roundld(sy, rep, q)

        @block.scalar
        def _(sc):
            sc.wait_ge(s_ini, 48)
            for rep in range(nrep):
                if rep > 0:
                    sc.wait_ge(s_f, rep)
                # phase A casts
                for b in range(NBA):
                    sc.wait_ge(s_xt, 16 * (rep * NBA + b + 1))
                    if b >= 2:
                        sc.wait_ge(s_pa, 4 * (rep * NBA + b - 1))
                    sc.activation(xb[b % 2][0:C, :], xs[b % 4][0:C, :],
                                  mybir.ActivationFunctionType.Copy
                                  ).then_inc(s_cx, 1)
                # phase B edge loads + output stores
                for pp in range(min(3, NI)):
                    loads(sc, rep, pp)
                for p in range(NI):
                    gp_ = rep * NI + p
                    if p + 3 < NI:
                        sc.wait_ge(s_gg, 2 * (gp_ + 1))
                        sc.wait_ge(s_pp, 2 * (gp_ + 1))
                        loads(sc, rep, p + 3)
                    if (p + 1) % 4 == 0 or p == NI - 1:
                        f = p // 4
                        sc.wait_ge(s_yv, rep * NF + f + 1)
                        sc.dma_start(ydev[f], yt[:]).then_inc(s_yd, 16)

        @block.tensor
        def _(te):
            te.wait_ge(s_ini, 48)
            for rep in range(nrep):
                if rep > 0:
                    te.wait_ge(s_f, rep)
                for b in range(NBA):
                    te.wait_ge(s_cx, rep * NBA + b + 1)
                    if b >= 2:
                        te.wait_ge(s_tc, rep * NBA + b - 1)
                    for m in range(4):
                        te.matmul(psA[b % 2][32 * m:32 * m + 32, :],
                                  wt[0:C, :],
                                  xb[b % 2][0:C, m * 512:(m + 1) * 512],
                                  start=True, stop=True,
                                  tile_position=(0, 32 * m)).then_inc(s_pa, 1)
                for p in range(NI):
                    gp_ = rep * NI + p
                    te.wait_ge(s_pp, 2 * (gp_ + 1))
                    f = p // 4
                    if p % 4 == 0 and f >= 2:
                        te.wait_ge(s_yv, rep * NF + f - 1)
                    bank = f % 2
                    last = None
                    for uu in range(8):
                        o = (p % 4) * 128 + uu * 16
                        te.matmul(psB[bank][:, o:o + 8],
                                  pr[:, uu * 128:(uu + 1) * 128],
                                  mk[:, 0:8], start=True, stop=True)
                        last = te.matmul(psB[bank][:, o + 8:o + 16],
                                         pc[:, uu * 128:(uu + 1) * 128],
                                         mk[:, 8:16], start=True, stop=True)
                    last.then_inc(s_mm, 1)

        @block.vector
        def _(ve):
            ve.wait_ge(s_ini, 48)
            for rep in range(nrep):
                if rep > 0:
                    ve.wait_ge(s_f, rep)
                ve.memset(tab[:], 0.0).then_inc(s_zt, 1)
                for b in range(NBA):
                    ve.wait_ge(s_pa, 4 * (rep * NBA + b) + 4)
                    if b >= 2:
                        ve.wait_ge(s_tw, 64 * (rep * NBA + b - 1))
                    ve.tensor_copy(tstg[b % 2][:],
                                   psA[b % 2][:]).then_inc(s_tc, 1)
                for p in range(NI):
                    gp_ = rep * NI + p
                    ve.wait_ge(s_gg, 2 * (gp_ + 1))
                    ve.wait_ge(s_ix, 48 * (gp_ + 1))
                    if p >= 1:
                        ve.wait_ge(s_mm, gp_)
                    ve.tensor_tensor(out=pr[:], in0=gr[p % 2][:],
                                     in1=sfb[p % 3][:],
                                     op=mybir.AluOpType.mult).then_inc(s_pp, 1)
                    ve.tensor_tensor(out=pc[:], in0=gc[p % 2][:],
                                     in1=sfb[p % 3][:],
                                     op=mybir.AluOpType.mult).then_inc(s_pp, 1)
                    if (p + 1) % 4 == 0 or p == NI - 1:
                        f = p // 4
                        ve.wait_ge(s_mm, rep * NI + min(4 * (f + 1), NI))
                        if f >= 1:
                            ve.wait_ge(s_yd, 16 * (rep * NF + f))
                        ve.tensor_reduce(
                            out=sg[:].rearrange("p (pb u g) -> p pb u g",
                                                pb=4, u=8),
                            in_=psB[f % 2][:].rearrange(
                                "p (pb u t g) -> p pb u g t",
                                pb=4, u=8, t=2, g=8),
                            axis=mybir.AxisListType.X,
                            op=mybir.AluOpType.add)
                        ve.scalar_tensor_tensor(
                            out=ys[:], in0=sg[:], scalar=NEG,
                            in1=sg[:], op0=mybir.AluOpType.mult,
                            op1=mybir.AluOpType.max)
                        ve.tensor_tensor(
                            out=yt[:].rearrange("p (k h) -> p k h", h=H),
                            in0=ys[:].unsqueeze(2).broadcast_to([128, 256, H]),
                            in1=gm[:].unsqueeze(1).broadcast_to([128, 256, H]),
                            op=mybir.AluOpType.mult).then_inc(s_yv, 1)

        @block.gpsimd
        def _(gp):
            gp.load_library(apg_lib)
            for rep in range(nrep):
                if rep > 0:
                    gp.wait_ge(s_f, rep)
                q = 0
                for p in range(NI):
                    gp_ = rep * NI + p
                    while q < NRND and p == int(rbase[q]):
                        gp.wait_ge(s_tb, 256 * (rep * NRND + q) + 256)
                        q += 1
                    gp.wait_ge(s_ix, 48 * (gp_ + 1))
                    if p >= 2:
                        gp.wait_ge(s_pp, 2 * (gp_ - 1))
                    gp.ap_gather(
                        gr[p % 2][:].rearrange("p (k d) -> p k d", d=1),
                        tab[:].rearrange("p (n d) -> p n d", d=1),
                        idr[p % 3][:], 128, CH, 1, K).then_inc(s_gg, 1)
                    gp.ap_gather(
                        gc[p % 2][:].rearrange("p (k d) -> p k d", d=1),
                        tab[:].rearrange("p (n d) -> p n d", d=1),
                        idc[p % 3][:], 128, CH, 1, K).then_inc(s_gg, 1)

    nc.compile()
    return nc


def plan_rounds(cnts):
    """cnts [NCORES, 16] -> (tiles, rounds): 8 largest buckets (by max core
    count) in round 0, rest in round 1."""
    mx = cnts.max(axis=0)
    order = np.argsort(-mx, kind='stable')
    rounds = [list(int(b) for b in order[0:8]),
              list(int(b) for b in order[8:16])]
    tiles = [int(max(1, -(-int(mx[list(r)].max()) // K)))
             for r in rounds]
    return tiles, rounds


def prep_inputs3(x, edge_index, structural_features, layer_idx,
                 psi_w, psi_b, delta_w, u, gamma_h):
    import ml_dtypes
    li = int(layer_idx)
    b0 = np.asarray(psi_b + delta_w[li] + u[li], np.float32)      # [128]
    psi_w = np.asarray(psi_w, np.float32)
    W = np.zeros((C, 16), dtype=np.float32)
    W[:, 0:4] = psi_w[0:C]
    W[:, 4] = b0[0:C]
    W[:, 5:9] = psi_w[C:2 * C]
    W[:, 9] = b0[C:2 * C]
    wext = np.concatenate([W, W], axis=1).astype(ml_dtypes.bfloat16)
    mks = np.zeros((128, 16), dtype=np.float32)
    for g in range(8):
        mks[16 * g:16 * g + 5, g] = 1.0
        mks[16 * g + 5:16 * g + 10, 8 + g] = 1.0
    mks = mks.astype(ml_dtypes.bfloat16)
    gma = np.tile(np.asarray(gamma_h[li], np.float32)[None, :], (128, 1))
    xT = np.zeros((C, NP3), dtype=np.float32)
    xT[:, :N] = np.asarray(x, np.float32).T

    row = np.asarray(edge_index[0], np.int64)
    col = np.asarray(edge_index[1], np.int64)
    sfp = np.concatenate([np.asarray(structural_features, np.float32),
                          np.ones((E, 1), np.float32)], axis=1)      # [E, 5]

    cb = np.asarray(CB[1:4], np.int64)
    rch = np.searchsorted(cb, row, side='right')
    cch = np.searchsorted(cb, col, side='right')
    cbase = np.asarray(CB[:4], np.int64)
    bucket = rch * NCH + cch
    orders, cnts = [], np.zeros((NCORES, 16), dtype=np.int64)
    for c in range(NCORES):
        sl = slice(c * EC, (c + 1) * EC)
        orders.append(np.argsort(bucket[sl], kind='stable') + c * EC)
        cnts[c] = np.bincount(bucket[sl], minlength=16)
    tiles, rounds = plan_rounds(cnts)
    NI = sum(tiles)
    NF = (NI + 3) // 4

    in_maps, eid_all = [], []
    for c in range(NCORES):
        order = orders[c]
        boff = np.concatenate([[0], np.cumsum(cnts[c])])
        ids = np.full((NI, 8, K), -1, dtype=np.int64)
        for qr in range(2):
            p0 = int(0 if qr == 0 else tiles[0])
            for g in range(8):
                b = int(rounds[qr][g])
                lo, hi = int(boff[b]), int(boff[b + 1])
                buf = np.full(tiles[qr] * K, -1, dtype=np.int64)
                buf[:hi - lo] = order[lo:hi]
                ids[p0:p0 + tiles[qr], g, :] = buf.reshape(tiles[qr], K)
        valid = ids >= 0
        idsz = np.where(valid, ids, 0)
        rl = np.where(valid, row[idsz] - cbase[rch[idsz]], 0).astype(np.int16)
        cl = np.where(valid, col[idsz] - cbase[cch[idsz]], 0).astype(np.int16)

        def wrap(a):   # [NI, 8, K] -> [NI, 128, K//16]
            return np.ascontiguousarray(
                a.reshape(NI, 8, K // 16, 16).transpose(0, 1, 3, 2)
                .reshape(NI, 128, K // 16))
        sfv = np.where(valid[..., None], sfp[idsz], 0.0)   # [NI, 8, K, 5]
        sftd = np.zeros((NI, 8, 16, K), dtype=np.float32)
        sftd[:, :, 0:5, :] = sfv.transpose(0, 1, 3, 2)
        sftd[:, :, 5:10, :] = sfv.transpose(0, 1, 3, 2)
        in_maps.append({
            "xt_in": xT, "wext": wext, "mks": mks, "gma": gma,
            "idxr": wrap(rl), "idxc": wrap(cl),
            "sft": np.ascontiguousarray(sftd.reshape(NI, 128, K)),
        })
        # edge (p, g, u*128 + e_p) -> yd[f=p//4, e_p, (p%4, u, g)]
        NIp = NF * 4
        idsp = np.full((NIp, 8, K), -1, dtype=np.int64)
        idsp[:NI] = ids
        eo = (idsp.reshape(NF, 4, 8, 8, 128)      # [f, pb, g, u, e_p]
              .transpose(0, 4, 1, 3, 2))          # [f, e_p, pb, u, g]
        eid_all.append(np.ascontiguousarray(eo))
    return in_maps, eid_all, (tiles, rounds)


def unshard3(results, eid_all):
    y = np.empty((E, H), dtype=np.float32)
    for c in range(NCORES):
        yd = np.asarray(results[c]["ydev"])       # [NF, 128, 2048]
        ids = eid_all[c].reshape(-1)              # [f, e_p, pb, u, g]
        blk = yd.reshape(-1, H)
        v = ids >= 0
        y[ids[v]] = blk[v]
    return y


_CACHE = {}


def kernel(**inputs):
    in_maps, eid_all, (tiles, rounds) = prep_inputs3(**inputs)
    key = (tuple(tiles), tuple(tuple(r) for r in rounds))
    if key not in _CACHE:
        _CACHE[key] = build_program3(tiles, rounds)
    res = run_bass_kernel_spmd(_CACHE[key], in_maps,
                               core_ids=list(range(NCORES)))
    return unshard3(res.results, eid_all)
